# revision 2
# baseline (speedup 1.0000x reference)
"""Chunked non-uniform DFT on 8 Trainium2 NeuronCores (Bass/Tile).

vis[b,k] = sum_p exp(-2pi*i*(u_k*l_p + v_k*m_p + w_k*(n_p-1))) * sky[b,p]

Strategy (per core, visibilities sharded 8 ways => V_local = 2048):
  - t[p,k] = l_p*u_k + m_p*v_k + (n_p-1)*w_k  via K=3 fp32 PE matmul
    (pixels on partitions, vis on free dim), accumulated in PSUM.
  - r = t - round(t) in [-0.5, 0.5] via DVE magic-number round + subtract.
  - S = sin(2*pi*r) and C = sin(pi/2 - 2*pi*|r|) = cos(2*pi*t) on ACT
    (both arguments stay inside the Sin table domain [-pi, pi]).
  - vis partial sums: PE matmuls with sky (4 columns: R0,R1,I0,I1) as the
    stationary operand and S/C as moving operands, col-tiled so the S and C
    products use different PE column groups and accumulate into different
    PSUM partitions (0-3 and 32-35) of the same banks.
  - Host combines: vis_r[b] = C.R_b + S.I_b ; vis_i[b] = C.I_b - S.R_b
    (e^{i*phase} = cos(2 pi t) - i sin(2 pi t)).
"""

import numpy as np

B = 2
P = 16384          # pixels
V = 16384          # visibilities (total)
N_CORES = 8
VL = V // N_CORES  # 2048 per core

MAGIC = float(1.5 * 2**23)
TWO_PI = float(2.0 * np.pi)
HALF_PI = float(0.5 * np.pi)

PIX_CHUNK = 128            # pixels per chunk (partition dim)
N_PC = P // PIX_CHUNK      # 128 chunks
GROUP = 2                  # pix-chunks per ACT group (ACT FD = GROUP*VL)
T_FD = 1024                # free dim of one PSUM t-tile (2 banks)
MM_N = 512                 # matmul free dim (one PSUM bank)

_COMPILED = None


def _build():
    import concourse.bacc as bacc
    import concourse.mybir as mybir
    import concourse.tile as tile
    from concourse.alu_op_type import AluOpType

    nc = bacc.Bacc("TRN2", target_bir_lowering=False, debug=False,
                   num_devices=N_CORES)
    f32 = mybir.dt.float32
    u32 = mybir.dt.uint32

    lmn_d = nc.dram_tensor("lmn", [3, P], f32, kind="ExternalInput")
    uvw_d = nc.dram_tensor("uvw", [3, VL], f32, kind="ExternalInput")
    sky4_d = nc.dram_tensor("sky4", [PIX_CHUNK, N_PC * 4], f32,
                            kind="ExternalInput")
    out_d = nc.dram_tensor("out8", [8, VL], f32, kind="ExternalOutput")

    with tile.TileContext(nc) as tc:
        with (
            tc.tile_pool(name="const", bufs=1) as constp,
            tc.tile_pool(name="inp", bufs=1) as inp,
            tc.tile_pool(name="lmns", bufs=2) as lmnp,
            tc.tile_pool(name="kt", bufs=3) as kp,
            tc.tile_pool(name="rt", bufs=2) as rp,
            tc.tile_pool(name="rat", bufs=2) as rap,
            tc.tile_pool(name="st", bufs=2) as sp,
            tc.tile_pool(name="ct", bufs=2) as cp,
            tc.tile_pool(name="outs", bufs=1) as outp,
            tc.tile_pool(name="tps", bufs=2, space="PSUM") as tpsp,
            tc.tile_pool(name="vps", bufs=1, space="PSUM") as vpsp,
        ):
            halfpi_t = constp.tile([128, 1], f32)
            nc.vector.memset(halfpi_t[:], HALF_PI)

            uvw_t = inp.tile([3, VL], f32)
            nc.sync.dma_start(uvw_t[:], uvw_d[:])
            sky4_t = inp.tile([PIX_CHUNK, N_PC * 4], f32)
            nc.sync.dma_start(sky4_t[:], sky4_d[:])

            # vis accumulators: S products on partitions 0-3, C products on
            # partitions 32-35, same 4 PSUM banks.
            vis_ps = vpsp.tile([36, VL], f32)

            LMN_TILE = 8 * PIX_CHUNK  # stream lmn in [3, 1024] tiles
            n_lmn = P // LMN_TILE

            lmn_tiles = {}
            GFD = GROUP * VL  # ACT/abs free dim per group

            for g in range(N_PC // GROUP):
                r_t = rp.tile([128, GFD], f32)
                ra_t = rap.tile([128, GFD], f32)
                s_t = sp.tile([128, GFD], f32)
                c_t = cp.tile([128, GFD], f32)

                for h in range(GROUP):
                    pc = g * GROUP + h
                    li = pc // 8
                    if li not in lmn_tiles:
                        lt = lmnp.tile([3, LMN_TILE], f32, tag="lmn")
                        nc.sync.dma_start(
                            lt[:], lmn_d[:, li * LMN_TILE:(li + 1) * LMN_TILE])
                        lmn_tiles = {li: lt}
                    lmn_sl = lmn_tiles[li][:, (pc % 8) * PIX_CHUNK:
                                           (pc % 8 + 1) * PIX_CHUNK]

                    for q in range(VL // T_FD):
                        t_ps = tpsp.tile([128, T_FD], f32)
                        for n in range(T_FD // MM_N):
                            vs = q * T_FD + n * MM_N
                            nc.tensor.matmul(
                                t_ps[:, n * MM_N:(n + 1) * MM_N],
                                lmn_sl,
                                uvw_t[:, vs:vs + MM_N],
                                start=True, stop=True,
                            )
                        k_t = kp.tile([128, T_FD], f32)
                        nc.vector.tensor_scalar(
                            k_t[:], t_ps[:], MAGIC, MAGIC,
                            op0=AluOpType.add, op1=AluOpType.subtract)
                        nc.vector.tensor_tensor(
                            r_t[:, h * VL + q * T_FD: h * VL + (q + 1) * T_FD],
                            t_ps[:], k_t[:], op=AluOpType.subtract)

                nc.vector.tensor_scalar(
                    ra_t[:].bitcast(u32), r_t[:].bitcast(u32),
                    0x7FFFFFFF, None, op0=AluOpType.bitwise_and)
                nc.scalar.activation(
                    s_t[:], r_t[:], mybir.ActivationFunctionType.Sin,
                    bias=0.0, scale=TWO_PI)
                nc.scalar.activation(
                    c_t[:], ra_t[:], mybir.ActivationFunctionType.Sin,
                    bias=halfpi_t[:], scale=-TWO_PI)

                for h in range(GROUP):
                    pc = g * GROUP + h
                    sky_sl = sky4_t[:, pc * 4:(pc + 1) * 4]
                    start = pc == 0
                    stop = pc == N_PC - 1
                    for n in range(VL // MM_N):
                        nc.tensor.matmul(
                            vis_ps[0:4, n * MM_N:(n + 1) * MM_N],
                            sky_sl,
                            s_t[:, h * VL + n * MM_N: h * VL + (n + 1) * MM_N],
                            start=start, stop=stop, tile_position=(0, 0),
                        )
                        nc.tensor.matmul(
                            vis_ps[32:36, n * MM_N:(n + 1) * MM_N],
                            sky_sl,
                            c_t[:, h * VL + n * MM_N: h * VL + (n + 1) * MM_N],
                            start=start, stop=stop, tile_position=(0, 32),
                        )

            out_t = outp.tile([36, VL], f32)
            nc.scalar.copy(out_t[0:4, :], vis_ps[0:4, :])
            nc.scalar.copy(out_t[32:36, :], vis_ps[32:36, :])
            nc.sync.dma_start(out_d[0:4, :], out_t[0:4, :])
            nc.sync.dma_start(out_d[4:8, :], out_t[32:36, :])

    nc.compile()
    return nc


def kernel(sky_real, sky_imag, l_coords, m_coords, n_coords,
           u_coords, v_coords, w_coords):
    global _COMPILED
    from concourse.bass_utils import run_bass_kernel_spmd

    if _COMPILED is None:
        _COMPILED = _build()
    nc = _COMPILED

    lmn = np.ascontiguousarray(
        np.stack([l_coords, m_coords, n_coords - 1.0]).astype(np.float32))
    # sky4[pix, j] with j in {R0, R1, I0, I1}; laid out per pix-chunk:
    # SBUF tile [128, N_PC*4] where column pc*4+j holds chunk pc, family j.
    sky4 = np.stack([sky_real[0], sky_real[1], sky_imag[0], sky_imag[1]],
                    axis=1).astype(np.float32)          # [P, 4]
    sky4 = sky4.reshape(N_PC, PIX_CHUNK, 4).transpose(1, 0, 2).reshape(
        PIX_CHUNK, N_PC * 4)
    sky4 = np.ascontiguousarray(sky4)

    in_maps = []
    for c in range(N_CORES):
        sl = slice(c * VL, (c + 1) * VL)
        uvw = np.ascontiguousarray(
            np.stack([u_coords[sl], v_coords[sl], w_coords[sl]])
            .astype(np.float32))
        in_maps.append({"lmn": lmn, "uvw": uvw, "sky4": sky4})

    res = run_bass_kernel_spmd(nc, in_maps, core_ids=list(range(N_CORES)))

    vis = np.empty((B, V), dtype=np.complex64)
    for c in range(N_CORES):
        sl = slice(c * VL, (c + 1) * VL)
        o = res.results[c]["out8"]  # rows: SR0, SR1, SI0, SI1, CR0, CR1, CI0, CI1
        sr0, sr1, si0, si1, cr0, cr1, ci0, ci1 = o
        vis[0, sl] = (cr0 + si0) + 1j * (ci0 - sr0)
        vis[1, sl] = (cr1 + si1) + 1j * (ci1 - sr1)
    return vis
